# revision 11
# baseline (speedup 1.0000x reference)
"""MoE router kernel for Trainium2 (8 NeuronCores, SPMD data-parallel).

Strategy: shard the 8192 tokens across 8 cores (1024 tokens each) and
replicate the experts.  Each core computes the router softmax + top-2
combine weights exactly in fp32, then runs the dense grouped-GEMM
(8 experts x [1024 x 1024 x 1024]) in fp32r (TF32-like, ~1e-4 rel err)
with per-expert PSUM eviction fused as acc = psum * comb_e + acc on the
vector engine.  Expert-bias combine is a tiny [8]-contraction matmul.
Diversity / utilization / load-balance are computed as per-core partial
sums on device (GpSimd + ACT, hidden under the matmuls) and finished on
host (O(E^2) scalar work).
"""

import numpy as np
import jax
from jax.experimental.shard_map import shard_map
from jax.sharding import Mesh, PartitionSpec

import concourse.bass as bass
import concourse.mybir as mybir
import concourse.tile as tile
from concourse import bacc, bass2jax
from concourse.masks import make_identity

dt = mybir.dt
F32 = dt.float32
F32R = dt.float32r
ALU = mybir.AluOpType
ACTF = mybir.ActivationFunctionType
AX = mybir.AxisListType.X

B, S, D, F, E = 4, 2048, 1024, 1024, 8
N_CORES = 8
T = B * S                 # 8192 tokens
TPC = T // N_CORES        # 1024 tokens per core
NT = TPC // 128           # 8 token tiles per core
ND = D // 128             # 8 contraction chunks
NF = F // 512             # 2 free chunks
PAIRS = [(i, j) for i in range(E) for j in range(i + 1, E)]   # 28
FLAT_P = F * D + 2 * F    # 1050624 flattened params per expert
DIV_Q = FLAT_P // N_CORES // 128   # 1026 free elems per partition per core

_PROGRAM_CACHE: dict = {}
_RUNNER_CACHE: dict = {}


class _Runner:
    """Persistent jitted SPMD executor for a finalized bass program.

    Mirrors concourse.bass2jax.run_bass_via_pjrt but keeps the jitted
    callable so repeated invocations skip retracing/recompiling.
    """

    def __init__(self, nc, n_cores):
        bass2jax.install_neuronx_cc_hook()
        assert nc.dbg_addr is None or not nc.dbg_callbacks
        self.n_cores = n_cores
        partition_name = (nc.partition_id_tensor.name
                          if nc.partition_id_tensor else None)
        in_names, out_names, out_avals, zero_outs = [], [], [], []
        for alloc in nc.m.functions[0].allocations:
            if not isinstance(alloc, mybir.MemoryLocationSet):
                continue
            name = alloc.memorylocations[0].name
            if alloc.kind == "ExternalInput":
                if name != partition_name and name != (
                        nc.dbg_addr.name if nc.dbg_addr else None):
                    in_names.append(name)
            elif alloc.kind == "ExternalOutput":
                shape = tuple(alloc.tensor_shape)
                np_dt = mybir.dt.np(alloc.dtype)
                out_names.append(name)
                out_avals.append(jax.core.ShapedArray(shape, np_dt))
                zero_outs.append(np.zeros(shape, np_dt))
        self.in_names = in_names
        self.out_names = out_names
        self.out_avals = out_avals
        self.zero_outs = zero_outs
        n_params = len(in_names)
        n_outs = len(out_names)
        self.n_params = n_params
        all_in_names = list(in_names) + list(out_names)
        if nc.dbg_addr is not None:
            all_in_names.append(nc.dbg_addr.name)
        if partition_name is not None:
            all_in_names.append(partition_name)
        self.has_dbg = nc.dbg_addr is not None
        donate = tuple(range(n_params, n_params + n_outs))

        assert nc.dbg_addr is None, "build programs with debug=False"

        def _body(*args):
            operands = list(args)
            if partition_name is not None:
                operands.append(bass2jax.partition_id_tensor())
            outs = bass2jax._bass_exec_p.bind(
                *operands,
                out_avals=tuple(out_avals),
                in_names=tuple(all_in_names),
                out_names=tuple(out_names),
                lowering_input_output_aliases=(),
                sim_require_finite=True,
                sim_require_nnan=True,
                nc=nc,
            )
            return tuple(outs)

        devices = jax.devices()[:n_cores]
        mesh = Mesh(np.asarray(devices), ("core",))
        self.mesh = mesh
        in_specs = (PartitionSpec("core"),) * (n_params + n_outs)
        out_specs = (PartitionSpec("core"),) * n_outs
        self.sharded = jax.jit(
            shard_map(_body, mesh=mesh, in_specs=in_specs,
                      out_specs=out_specs, check_rep=False),
            donate_argnums=donate, keep_unused=True)

    def device_put_inputs(self, concat_in):
        sh = jax.sharding.NamedSharding(self.mesh, PartitionSpec("core"))
        return [jax.device_put(a, sh) for a in concat_in]

    def device_put_zeros(self):
        sh = jax.sharding.NamedSharding(self.mesh, PartitionSpec("core"))
        return [jax.device_put(z, sh) for z in self.zero_buffers()]

    def concat_inputs(self, in_maps):
        return [np.concatenate([np.asarray(m[n]) for m in in_maps], axis=0)
                for n in self.in_names]

    def zero_buffers(self):
        return [np.zeros((self.n_cores * z.shape[0], *z.shape[1:]), z.dtype)
                for z in self.zero_outs]

    def execute(self, concat_in):
        out = self.sharded(*concat_in, *self.zero_buffers())
        return jax.block_until_ready(out)

    def run(self, in_maps):
        out_arrs = self.execute(self.concat_inputs(in_maps))
        return [
            {name: np.asarray(out_arrs[i]).reshape(
                self.n_cores, *self.out_avals[i].shape)[c]
             for i, name in enumerate(self.out_names)}
            for c in range(self.n_cores)
        ]


def get_runner(repeat: int = 1) -> _Runner:
    if repeat not in _RUNNER_CACHE:
        if repeat not in _PROGRAM_CACHE:
            _PROGRAM_CACHE[repeat] = build_program(repeat)
        _RUNNER_CACHE[repeat] = _Runner(_PROGRAM_CACHE[repeat], N_CORES)
    return _RUNNER_CACHE[repeat]


def _emit_body(nc, tc, io):
    (xT_d, xTf_d, rwT_d, rb_d, eb_d, divs_d, wT_d,
     out_d, rw_d, up_d, dv_d) = io

    with (
        tc.tile_pool(name="const", bufs=1) as constp,
        tc.tile_pool(name="wpool", bufs=2) as wpool,
        tc.tile_pool(name="work", bufs=3) as work,
    ):
        xT = constp.tile([128, ND, TPC], F32R)
        nc.sync.dma_start(xT[:], xT_d[:])
        rwT = constp.tile([128, ND, E], F32)
        nc.sync.dma_start(rwT[:], rwT_d[:])
        rb = constp.tile([1, E], F32)
        nc.sync.dma_start(rb[:], rb_d[:])
        eb = constp.tile([E, F], F32R)
        nc.sync.dma_start(eb[:], eb_d[:])
        div_sb = constp.tile([128, E, DIV_Q], F32)
        nc.sync.dma_start(div_sb[:], divs_d[:])

        ident = constp.tile([128, 128], F32)
        make_identity(nc, ident[:])
        ones1 = constp.tile([1, 128], F32)
        nc.vector.memset(ones1[:], 1.0)

        comb_all = constp.tile([128, NT, E], F32)
        combT_all = constp.tile([E, NT, 128], F32R)
        acc_all = constp.tile([128, NT, F], F32)
        usage = constp.tile([128, E], F32)
        util = constp.tile([128, E], F32)
        dv_sb = constp.tile([128, len(PAIRS)], F32)
        nc.vector.memset(usage[:], 0.0)
        nc.vector.memset(util[:], 0.0)

        # ---- Phase 1: router (exact fp32) + softmax + top-2 combine ----
        with (
            tc.tile_pool(name="ps1", bufs=2, space="PSUM") as ps1,
            tc.tile_pool(name="ps1t", bufs=2, space="PSUM") as ps1t,
        ):
            for i in range(NT):
                tsl = slice(i * 128, (i + 1) * 128)
                # stream an exact fp32 slice of x^T for the router matmul
                # (the resident f32r copy is TF32-rounded by the DMA write,
                # which would perturb top-2 selection near ties)
                xtile = work.tile([128, ND, 128], F32)
                nc.sync.dma_start(xtile[:], xTf_d[:, :, tsl])
                lg = ps1.tile([128, E], F32)
                for c in range(ND):
                    nc.tensor.matmul(lg[:], xtile[:, c, :],
                                     rwT[:, c, :], start=(c == 0), stop=False)
                nc.tensor.matmul(lg[:], ones1[:], rb[:], start=False, stop=True)

                # top-2 mask from EXACT fp32 logits (softmax is monotone, so
                # the selection matches the reference; the LUT-based exp only
                # perturbs the smooth renorm weights, never the selection).
                lgs = work.tile([128, E], F32)
                nc.vector.tensor_copy(lgs[:], lg[:])
                top8 = work.tile([128, 8], F32)
                nc.vector.max(top8[:], lgs[:])
                mask = work.tile([128, E], F32)
                nc.vector.tensor_scalar(mask[:], lgs[:], top8[:, 1:2], None,
                                        ALU.is_ge)
                nc.vector.tensor_tensor(util[:], util[:], mask[:], ALU.add)

                negmax = work.tile([128, 1], F32)
                nc.vector.tensor_scalar_mul(negmax[:], top8[:, 0:1], -1.0)
                expv = work.tile([128, E], F32)
                nc.scalar.activation(expv[:], lg[:], ACTF.Exp,
                                     bias=negmax[:], scale=1.0)
                ssum = work.tile([128, 1], F32)
                nc.vector.reduce_sum(ssum[:], expv[:], AX)
                rinv = work.tile([128, 1], F32)
                nc.vector.reciprocal(rinv[:], ssum[:])
                rwv = work.tile([128, E], F32)
                nc.vector.tensor_scalar_mul(rwv[:], expv[:], rinv[:])
                nc.sync.dma_start(rw_d[tsl, :], rwv[:])
                nc.vector.tensor_tensor(usage[:], usage[:], rwv[:], ALU.add)

                # masked rw + its row-sum in one pass, then renormalize
                mrw = work.tile([128, E], F32)
                den = work.tile([128, 1], F32)
                nc.vector.scalar_tensor_tensor(mrw[:], rwv[:], 1.0, mask[:],
                                               ALU.mult, ALU.mult,
                                               accum_out=den[:])
                rden = work.tile([128, 1], F32)
                nc.vector.reciprocal(rden[:], den[:])
                nc.vector.tensor_scalar_mul(comb_all[:, i, :], mrw[:],
                                            rden[:])

                tp = ps1t.tile([E, 128], F32)
                nc.tensor.transpose(tp[:], comb_all[:, i, :], ident[:])
                nc.scalar.copy(combT_all[:, i, :], tp[:])

        # ---- acc init: bias combine  acc[t,f] = sum_e comb[t,e] b[e,f] ----
        with tc.tile_pool(name="ps2", bufs=2, space="PSUM") as ps2:
            for i in range(NT):
                for f2 in range(NF):
                    fsl = slice(f2 * 512, (f2 + 1) * 512)
                    bp = ps2.tile([128, 512], F32)
                    nc.tensor.matmul(bp[:], combT_all[:, i, :], eb[:, fsl],
                                     start=True, stop=True)
                    nc.scalar.copy(acc_all[:, i, fsl], bp[:])

        # ---- diversity partials (GpSimd sub + ACT square-accumulate) ----
        for p_idx, (a, b) in enumerate(PAIRS):
            dtmp = work.tile([128, DIV_Q], F32)
            nc.gpsimd.tensor_tensor(dtmp[:], div_sb[:, a, :], div_sb[:, b, :],
                                    ALU.subtract)
            dsq = work.tile([128, DIV_Q], F32)
            nc.scalar.activation(dsq[:], dtmp[:], ACTF.Square,
                                 accum_out=dv_sb[:, p_idx:p_idx + 1])

        # ---- Phase 2: dense expert GEMMs (fp32r) + fused combine ----
        with tc.tile_pool(name="ps3", bufs=4, space="PSUM") as ps3:
            for e in range(E):
                wTe = wpool.tile([128, ND, F], F32R)
                nc.sync.dma_start(wTe[:], wT_d[e])
                for i in range(NT):
                    tsl = slice(i * 128, (i + 1) * 128)
                    for f2 in range(NF):
                        fsl = slice(f2 * 512, (f2 + 1) * 512)
                        pp = ps3.tile([128, 512], F32)
                        for c in range(ND):
                            nc.tensor.matmul(pp[:], xT[:, c, tsl],
                                             wTe[:, c, fsl],
                                             start=(c == 0), stop=(c == ND - 1))
                        nc.vector.scalar_tensor_tensor(
                            acc_all[:, i, fsl], pp[:], comb_all[:, i, e:e + 1],
                            acc_all[:, i, fsl], ALU.mult, ALU.add)

        # ---- outputs ----
        for i in range(NT):
            nc.sync.dma_start(out_d[i * 128:(i + 1) * 128, :], acc_all[:, i, :])
        nc.sync.dma_start(up_d[:, 0:E], usage[:])
        nc.sync.dma_start(up_d[:, E:2 * E], util[:])
        nc.sync.dma_start(dv_d[:], dv_sb[:])


def build_program(repeat: int = 1):
    nc = bacc.Bacc(None)
    xT_d = nc.declare_dram_parameter("xT", [128, ND, TPC], F32R, isOutput=False)
    xTf_d = nc.declare_dram_parameter("xTf", [128, ND, TPC], F32,
                                      isOutput=False)
    rwT_d = nc.declare_dram_parameter("rwT", [128, ND, E], F32, isOutput=False)
    rb_d = nc.declare_dram_parameter("rb", [1, E], F32, isOutput=False)
    eb_d = nc.declare_dram_parameter("eb", [E, F], F32R, isOutput=False)
    divs_d = nc.declare_dram_parameter("divs", [128, E, DIV_Q], F32,
                                       isOutput=False)
    wT_d = nc.declare_dram_parameter("wT", [E, 128, ND, F], F32R,
                                     isOutput=False)
    out_d = nc.declare_dram_parameter("out", [TPC, F], F32, isOutput=True)
    rw_d = nc.declare_dram_parameter("rw", [TPC, E], F32, isOutput=True)
    up_d = nc.declare_dram_parameter("up", [128, 2 * E], F32, isOutput=True)
    dv_d = nc.declare_dram_parameter("dv", [128, len(PAIRS)], F32,
                                     isOutput=True)
    io = (xT_d, xTf_d, rwT_d, rb_d, eb_d, divs_d, wT_d, out_d, rw_d, up_d,
          dv_d)

    with tile.TileContext(nc) as tc:
        if repeat > 1:
            with tc.For_i(0, repeat, 1):
                _emit_body(nc, tc, io)
        else:
            _emit_body(nc, tc, io)
    nc.finalize()
    return nc


def _stage_inputs(x, router_w, router_b, expert_w, expert_b, expert_pref):
    """Host-side layout staging shared by all cores."""
    X = np.ascontiguousarray(x.reshape(T, D), dtype=np.float32)
    xTf = np.ascontiguousarray(X.T)                       # [D, T]
    # [D, T] -> per-core [128, ND, TPC]
    xT_cores = []
    for c in range(N_CORES):
        sl = xTf[:, c * TPC:(c + 1) * TPC]                # [1024, 1024]
        xT_cores.append(np.ascontiguousarray(
            sl.reshape(ND, 128, TPC).transpose(1, 0, 2)))

    rwT = np.ascontiguousarray(
        router_w.T.reshape(ND, 128, E).transpose(1, 0, 2)).astype(np.float32)
    rb = np.ascontiguousarray(router_b.reshape(1, E), dtype=np.float32)

    # expert_w [E, F, D] -> wT [E, 128, ND, F]
    wT = np.ascontiguousarray(
        expert_w.transpose(0, 2, 1)          # [E, D, F]
        .reshape(E, ND, 128, F)
        .transpose(0, 2, 1, 3)).astype(np.float32)
    eb = np.ascontiguousarray(expert_b, dtype=np.float32)

    flat = np.concatenate(
        [expert_w.reshape(E, -1), expert_b, expert_pref], axis=1
    ).astype(np.float32)                                   # [E, FLAT_P]
    divs_cores = []
    per_core = FLAT_P // N_CORES                           # 131328
    for c in range(N_CORES):
        sl = flat[:, c * per_core:(c + 1) * per_core]      # [E, 131328]
        divs_cores.append(np.ascontiguousarray(
            sl.reshape(E, 128, DIV_Q).transpose(1, 0, 2)))

    return xT_cores, rwT, rb, wT, eb, divs_cores


def kernel(x, router_w, router_b, expert_w, expert_b, expert_pref, top_k,
           _repeat: int = 1):
    assert int(top_k) == 2
    x = np.asarray(x, dtype=np.float32)
    router_w = np.asarray(router_w, dtype=np.float32)
    router_b = np.asarray(router_b, dtype=np.float32)
    expert_w = np.asarray(expert_w, dtype=np.float32)
    expert_b = np.asarray(expert_b, dtype=np.float32)
    expert_pref = np.asarray(expert_pref, dtype=np.float32)

    xT_cores, rwT, rb, wT, eb, divs_cores = _stage_inputs(
        x, router_w, router_b, expert_w, expert_b, expert_pref)

    runner = get_runner(_repeat)
    in_maps = [
        {"xT": xT_cores[c], "xTf": xT_cores[c], "rwT": rwT, "rb": rb,
         "eb": eb, "divs": divs_cores[c], "wT": wT}
        for c in range(N_CORES)
    ]
    outs = runner.run(in_maps)
    final_output = np.concatenate([o["out"] for o in outs], axis=0) \
        .reshape(B, S, F)
    routing_weights = np.concatenate([o["rw"] for o in outs], axis=0) \
        .reshape(B, S, E)

    up = np.stack([o["up"] for o in outs])                 # [C, 128, 16]
    usage = up[:, :, :E].sum(axis=(0, 1)) / T
    util = up[:, :, E:].sum(axis=(0, 1)) / T
    load_balance_loss = np.float32(np.mean((usage - 1.0 / E) ** 2))
    expert_utilization = util.astype(np.float32)

    dv = np.stack([o["dv"] for o in outs])                 # [C, 128, 28]
    d2 = dv.sum(axis=(0, 1))                               # [28]
    diversity_score = np.float32(np.mean(np.sqrt(d2)))

    return (final_output.astype(np.float32),
            routing_weights.astype(np.float32),
            expert_utilization,
            load_balance_loss,
            diversity_score)


# revision 18
# speedup vs baseline: 1.0580x; 1.0580x over previous
"""MoE router kernel for Trainium2 (8 NeuronCores, SPMD data-parallel).

Strategy: shard the 8192 tokens across 8 cores (1024 tokens each) and
replicate the experts.  Each core computes the router softmax + top-2
combine weights exactly in fp32, then runs the dense grouped-GEMM
(8 experts x [1024 x 1024 x 1024]) in fp32r (TF32-like, ~1e-4 rel err)
with per-expert PSUM eviction fused as acc = psum * comb_e + acc on the
vector engine.  Expert-bias combine is a tiny [8]-contraction matmul.
Diversity / utilization / load-balance are computed as per-core partial
sums on device (GpSimd + ACT, hidden under the matmuls) and finished on
host (O(E^2) scalar work).
"""

import numpy as np
import jax
from jax.experimental.shard_map import shard_map
from jax.sharding import Mesh, PartitionSpec

import concourse.bass as bass
import concourse.mybir as mybir
import concourse.tile as tile
from concourse import bacc, bass2jax
from concourse.bass_utils import run_bass_kernel_spmd
from concourse.masks import make_identity

dt = mybir.dt
F32 = dt.float32
F32R = dt.float32r
ALU = mybir.AluOpType
ACTF = mybir.ActivationFunctionType
AX = mybir.AxisListType.X

B, S, D, F, E = 4, 2048, 1024, 1024, 8
N_CORES = 8
T = B * S                 # 8192 tokens
TPC = T // N_CORES        # 1024 tokens per core
NT = TPC // 128           # 8 token tiles per core
ND = D // 128             # 8 contraction chunks
NF = F // 512             # 2 free chunks
PAIRS = [(i, j) for i in range(E) for j in range(i + 1, E)]   # 28
FLAT_P = F * D + 2 * F    # 1050624 flattened params per expert
DIV_Q = FLAT_P // N_CORES // 128   # 1026 free elems per partition per core

_PROGRAM_CACHE: dict = {}
_RUNNER_CACHE: dict = {}


class _Runner:
    """Persistent jitted SPMD executor for a finalized bass program.

    Mirrors concourse.bass2jax.run_bass_via_pjrt but keeps the jitted
    callable so repeated invocations skip retracing/recompiling.
    """

    def __init__(self, nc, n_cores):
        bass2jax.install_neuronx_cc_hook()
        assert nc.dbg_addr is None or not nc.dbg_callbacks
        self.n_cores = n_cores
        partition_name = (nc.partition_id_tensor.name
                          if nc.partition_id_tensor else None)
        in_names, out_names, out_avals, zero_outs = [], [], [], []
        for alloc in nc.m.functions[0].allocations:
            if not isinstance(alloc, mybir.MemoryLocationSet):
                continue
            name = alloc.memorylocations[0].name
            if alloc.kind == "ExternalInput":
                if name != partition_name and name != (
                        nc.dbg_addr.name if nc.dbg_addr else None):
                    in_names.append(name)
            elif alloc.kind == "ExternalOutput":
                shape = tuple(alloc.tensor_shape)
                np_dt = mybir.dt.np(alloc.dtype)
                out_names.append(name)
                out_avals.append(jax.core.ShapedArray(shape, np_dt))
                zero_outs.append(np.zeros(shape, np_dt))
        self.in_names = in_names
        self.out_names = out_names
        self.out_avals = out_avals
        self.zero_outs = zero_outs
        n_params = len(in_names)
        n_outs = len(out_names)
        self.n_params = n_params
        all_in_names = list(in_names) + list(out_names)
        if nc.dbg_addr is not None:
            all_in_names.append(nc.dbg_addr.name)
        if partition_name is not None:
            all_in_names.append(partition_name)
        self.has_dbg = nc.dbg_addr is not None
        donate = tuple(range(n_params, n_params + n_outs))

        assert nc.dbg_addr is None, "build programs with debug=False"

        def _body(*args):
            operands = list(args)
            if partition_name is not None:
                operands.append(bass2jax.partition_id_tensor())
            outs = bass2jax._bass_exec_p.bind(
                *operands,
                out_avals=tuple(out_avals),
                in_names=tuple(all_in_names),
                out_names=tuple(out_names),
                lowering_input_output_aliases=(),
                sim_require_finite=True,
                sim_require_nnan=True,
                nc=nc,
            )
            return tuple(outs)

        devices = jax.devices()[:n_cores]
        mesh = Mesh(np.asarray(devices), ("core",))
        self.mesh = mesh
        in_specs = (PartitionSpec("core"),) * (n_params + n_outs)
        out_specs = (PartitionSpec("core"),) * n_outs
        self.sharded = jax.jit(
            shard_map(_body, mesh=mesh, in_specs=in_specs,
                      out_specs=out_specs, check_rep=False),
            donate_argnums=donate, keep_unused=True)

    def device_put_inputs(self, concat_in):
        sh = jax.sharding.NamedSharding(self.mesh, PartitionSpec("core"))
        return [jax.device_put(a, sh) for a in concat_in]

    def device_put_zeros(self):
        sh = jax.sharding.NamedSharding(self.mesh, PartitionSpec("core"))
        return [jax.device_put(z, sh) for z in self.zero_buffers()]

    def concat_inputs(self, in_maps):
        return [np.concatenate([np.asarray(m[n]) for m in in_maps], axis=0)
                for n in self.in_names]

    def zero_buffers(self):
        return [np.zeros((self.n_cores * z.shape[0], *z.shape[1:]), z.dtype)
                for z in self.zero_outs]

    def execute(self, concat_in):
        out = self.sharded(*concat_in, *self.zero_buffers())
        return jax.block_until_ready(out)

    def run(self, in_maps):
        out_arrs = self.execute(self.concat_inputs(in_maps))
        return [
            {name: np.asarray(out_arrs[i]).reshape(
                self.n_cores, *self.out_avals[i].shape)[c]
             for i, name in enumerate(self.out_names)}
            for c in range(self.n_cores)
        ]


def get_runner(repeat: int = 1, variant: str = "full") -> _Runner:
    key = (repeat, variant)
    if key not in _RUNNER_CACHE:
        if key not in _PROGRAM_CACHE:
            _PROGRAM_CACHE[key] = build_program(repeat, variant)
        _RUNNER_CACHE[key] = _Runner(_PROGRAM_CACHE[key], N_CORES)
    return _RUNNER_CACHE[key]


def _emit_body(nc, tc, io, variant="full"):
    (xT_d, xTf_d, rwT_d, rb_d, eb_d, divs_d, wT_d,
     out_d, rw_d, up_d, dv_d) = io

    with (
        tc.tile_pool(name="const", bufs=1) as constp,
        tc.tile_pool(name="wpool", bufs=2) as wpool,
        tc.tile_pool(name="work", bufs=3) as work,
    ):
        xT = constp.tile([128, ND, TPC], F32R)
        nc.sync.dma_start(xT[:], xT_d[:])
        rwT = constp.tile([128, ND, E], F32)
        nc.sync.dma_start(rwT[:], rwT_d[:])
        rb = constp.tile([1, E], F32)
        nc.sync.dma_start(rb[:], rb_d[:])
        eb = constp.tile([E, F], F32R)
        nc.sync.dma_start(eb[:], eb_d[:])
        div_sb = constp.tile([128, E, DIV_Q], F32)
        nc.sync.dma_start(div_sb[:], divs_d[:])

        ident = constp.tile([128, 128], F32)
        make_identity(nc, ident[:])
        ones1 = constp.tile([1, 128], F32)
        nc.vector.memset(ones1[:], 1.0)

        comb_all = constp.tile([128, NT, E], F32)
        combT_all = constp.tile([E, NT, 128], F32R)
        acc_all = constp.tile([128, NT, F], F32)
        usage = constp.tile([128, E], F32)
        util = constp.tile([128, E], F32)
        dv_sb = constp.tile([128, len(PAIRS)], F32)
        nc.vector.memset(usage[:], 0.0)
        nc.vector.memset(util[:], 0.0)

        # ---- Phase 1: router (exact fp32) + softmax + top-2 combine ----
        with (
            tc.tile_pool(name="ps1", bufs=2, space="PSUM") as ps1,
            tc.tile_pool(name="ps1t", bufs=2, space="PSUM") as ps1t,
        ):
            for i in range(NT):
                tsl = slice(i * 128, (i + 1) * 128)
                # stream an exact fp32 slice of x^T for the router matmul
                # (the resident f32r copy is TF32-rounded by the DMA write,
                # which would perturb top-2 selection near ties)
                xtile = work.tile([128, ND, 128], F32)
                nc.sync.dma_start(xtile[:], xTf_d[:, :, tsl])
                lg = ps1.tile([128, E], F32)
                for c in range(ND):
                    nc.tensor.matmul(lg[:], xtile[:, c, :],
                                     rwT[:, c, :], start=(c == 0), stop=False)
                nc.tensor.matmul(lg[:], ones1[:], rb[:], start=False, stop=True)

                # top-2 mask from EXACT fp32 logits (softmax is monotone, so
                # the selection matches the reference; the LUT-based exp only
                # perturbs the smooth renorm weights, never the selection).
                lgs = work.tile([128, E], F32)
                nc.vector.tensor_copy(lgs[:], lg[:])
                top8 = work.tile([128, 8], F32)
                nc.vector.max(top8[:], lgs[:])
                mask = work.tile([128, E], F32)
                nc.vector.tensor_scalar(mask[:], lgs[:], top8[:, 1:2], None,
                                        ALU.is_ge)
                nc.vector.tensor_tensor(util[:], util[:], mask[:], ALU.add)

                negmax = work.tile([128, 1], F32)
                nc.vector.tensor_scalar_mul(negmax[:], top8[:, 0:1], -1.0)
                expv = work.tile([128, E], F32)
                nc.scalar.activation(expv[:], lg[:], ACTF.Exp,
                                     bias=negmax[:], scale=1.0)
                ssum = work.tile([128, 1], F32)
                nc.vector.reduce_sum(ssum[:], expv[:], AX)
                rinv = work.tile([128, 1], F32)
                nc.vector.reciprocal(rinv[:], ssum[:])
                rwv = work.tile([128, E], F32)
                nc.vector.tensor_scalar_mul(rwv[:], expv[:], rinv[:])
                nc.sync.dma_start(rw_d[tsl, :], rwv[:])
                nc.vector.tensor_tensor(usage[:], usage[:], rwv[:], ALU.add)

                # masked rw + its row-sum in one pass, then renormalize
                mrw = work.tile([128, E], F32)
                den = work.tile([128, 1], F32)
                nc.vector.scalar_tensor_tensor(mrw[:], rwv[:], 1.0, mask[:],
                                               ALU.mult, ALU.mult,
                                               accum_out=den[:])
                rden = work.tile([128, 1], F32)
                nc.vector.reciprocal(rden[:], den[:])
                nc.vector.tensor_scalar_mul(comb_all[:, i, :], mrw[:],
                                            rden[:])

                tp = ps1t.tile([E, 128], F32)
                nc.tensor.transpose(tp[:], comb_all[:, i, :], ident[:])
                nc.scalar.copy(combT_all[:, i, :], tp[:])

        # ---- acc init: bias combine  acc[t,f] = sum_e comb[t,e] b[e,f] ----
        with tc.tile_pool(name="ps2", bufs=2, space="PSUM") as ps2:
            for i in range(NT):
                for f2 in range(NF):
                    fsl = slice(f2 * 512, (f2 + 1) * 512)
                    bp = ps2.tile([128, 512], F32)
                    nc.tensor.matmul(bp[:], combT_all[:, i, :], eb[:, fsl],
                                     start=True, stop=True)
                    nc.scalar.copy(acc_all[:, i, fsl], bp[:])

        # ---- diversity partials (GpSimd sub + ACT square-accumulate) ----
        for p_idx, (a, b) in enumerate(PAIRS):
            dtmp = work.tile([128, DIV_Q], F32)
            nc.gpsimd.tensor_tensor(dtmp[:], div_sb[:, a, :], div_sb[:, b, :],
                                    ALU.subtract)
            dsq = work.tile([128, DIV_Q], F32)
            nc.scalar.activation(dsq[:], dtmp[:], ACTF.Square,
                                 accum_out=dv_sb[:, p_idx:p_idx + 1])

        # ---- Phase 2: dense expert GEMMs (fp32r) + fused combine ----
        resident_w = []
        if variant == "pe_only":
            # only 2 distinct experts resident, no per-expert W DMA:
            # isolates PE+eviction time from the W-streaming DMA.
            for k in range(2):
                wre = wpool.tile([128, ND, F], F32R)
                nc.sync.dma_start(wre[:], wT_d[k])
                resident_w.append(wre)
        with tc.tile_pool(name="ps3", bufs=4, space="PSUM") as ps3:
            for e in range(E):
                if variant == "pe_only":
                    wTe = resident_w[e % 2]
                elif variant == "no_mm":
                    wTe = wpool.tile([128, ND, F], F32R)
                    nc.sync.dma_start(wTe[:], wT_d[e])
                    continue
                else:
                    wTe = wpool.tile([128, ND, F], F32R)
                    nc.sync.dma_start(wTe[:], wT_d[e])
                for i in range(NT):
                    tsl = slice(i * 128, (i + 1) * 128)
                    for f2 in range(NF):
                        fsl = slice(f2 * 512, (f2 + 1) * 512)
                        pp = ps3.tile([128, 512], F32)
                        for c in range(ND):
                            nc.tensor.matmul(pp[:], xT[:, c, tsl],
                                             wTe[:, c, fsl],
                                             start=(c == 0), stop=(c == ND - 1))
                        nc.vector.scalar_tensor_tensor(
                            acc_all[:, i, fsl], pp[:], comb_all[:, i, e:e + 1],
                            acc_all[:, i, fsl], ALU.mult, ALU.add)

        # ---- outputs ----
        for i in range(NT):
            nc.sync.dma_start(out_d[i * 128:(i + 1) * 128, :], acc_all[:, i, :])
        nc.sync.dma_start(up_d[:, 0:E], usage[:])
        nc.sync.dma_start(up_d[:, E:2 * E], util[:])
        nc.sync.dma_start(dv_d[:], dv_sb[:])


def build_program(repeat: int = 1, variant: str = "full"):
    nc = bacc.Bacc(None)
    xT_d = nc.declare_dram_parameter("xT", [128, ND, TPC], F32R, isOutput=False)
    xTf_d = nc.declare_dram_parameter("xTf", [128, ND, TPC], F32,
                                      isOutput=False)
    rwT_d = nc.declare_dram_parameter("rwT", [128, ND, E], F32, isOutput=False)
    rb_d = nc.declare_dram_parameter("rb", [1, E], F32, isOutput=False)
    eb_d = nc.declare_dram_parameter("eb", [E, F], F32R, isOutput=False)
    divs_d = nc.declare_dram_parameter("divs", [128, E, DIV_Q], F32,
                                       isOutput=False)
    wT_d = nc.declare_dram_parameter("wT", [E, 128, ND, F], F32R,
                                     isOutput=False)
    out_d = nc.declare_dram_parameter("out", [TPC, F], F32, isOutput=True)
    rw_d = nc.declare_dram_parameter("rw", [TPC, E], F32, isOutput=True)
    up_d = nc.declare_dram_parameter("up", [128, 2 * E], F32, isOutput=True)
    dv_d = nc.declare_dram_parameter("dv", [128, len(PAIRS)], F32,
                                     isOutput=True)
    io = (xT_d, xTf_d, rwT_d, rb_d, eb_d, divs_d, wT_d, out_d, rw_d, up_d,
          dv_d)

    with tile.TileContext(nc) as tc:
        if repeat > 1:
            with tc.For_i(0, repeat, 1):
                _emit_body(nc, tc, io, variant)
        else:
            _emit_body(nc, tc, io, variant)
    nc.finalize()
    return nc


def _stage_inputs(x, router_w, router_b, expert_w, expert_b, expert_pref):
    """Host-side layout staging shared by all cores."""
    X = np.ascontiguousarray(x.reshape(T, D), dtype=np.float32)
    xTf = np.ascontiguousarray(X.T)                       # [D, T]
    # [D, T] -> per-core [128, ND, TPC]
    xT_cores = []
    for c in range(N_CORES):
        sl = xTf[:, c * TPC:(c + 1) * TPC]                # [1024, 1024]
        xT_cores.append(np.ascontiguousarray(
            sl.reshape(ND, 128, TPC).transpose(1, 0, 2)))

    rwT = np.ascontiguousarray(
        router_w.T.reshape(ND, 128, E).transpose(1, 0, 2)).astype(np.float32)
    rb = np.ascontiguousarray(router_b.reshape(1, E), dtype=np.float32)

    # expert_w [E, F, D] -> wT [E, 128, ND, F]
    wT = np.ascontiguousarray(
        expert_w.transpose(0, 2, 1)          # [E, D, F]
        .reshape(E, ND, 128, F)
        .transpose(0, 2, 1, 3)).astype(np.float32)
    eb = np.ascontiguousarray(expert_b, dtype=np.float32)

    flat = np.concatenate(
        [expert_w.reshape(E, -1), expert_b, expert_pref], axis=1
    ).astype(np.float32)                                   # [E, FLAT_P]
    divs_cores = []
    per_core = FLAT_P // N_CORES                           # 131328
    for c in range(N_CORES):
        sl = flat[:, c * per_core:(c + 1) * per_core]      # [E, 131328]
        divs_cores.append(np.ascontiguousarray(
            sl.reshape(E, 128, DIV_Q).transpose(1, 0, 2)))

    return xT_cores, rwT, rb, wT, eb, divs_cores


def kernel(x, router_w, router_b, expert_w, expert_b, expert_pref, top_k,
           _repeat: int = 1, _use_persistent_runner: bool = True):
    assert int(top_k) == 2
    x = np.asarray(x, dtype=np.float32)
    router_w = np.asarray(router_w, dtype=np.float32)
    router_b = np.asarray(router_b, dtype=np.float32)
    expert_w = np.asarray(expert_w, dtype=np.float32)
    expert_b = np.asarray(expert_b, dtype=np.float32)
    expert_pref = np.asarray(expert_pref, dtype=np.float32)

    xT_cores, rwT, rb, wT, eb, divs_cores = _stage_inputs(
        x, router_w, router_b, expert_w, expert_b, expert_pref)

    in_maps = [
        {"xT": xT_cores[c], "xTf": xT_cores[c], "rwT": rwT, "rb": rb,
         "eb": eb, "divs": divs_cores[c], "wT": wT}
        for c in range(N_CORES)
    ]
    if _use_persistent_runner:
        outs = get_runner(_repeat).run(in_maps)
    else:
        key = (_repeat, "full")
        if key not in _PROGRAM_CACHE:
            _PROGRAM_CACHE[key] = build_program(_repeat)
        res = run_bass_kernel_spmd(_PROGRAM_CACHE[key], in_maps,
                                   list(range(N_CORES)))
        outs = [res.results[c] for c in range(N_CORES)]
    final_output = np.concatenate([o["out"] for o in outs], axis=0) \
        .reshape(B, S, F)
    routing_weights = np.concatenate([o["rw"] for o in outs], axis=0) \
        .reshape(B, S, E)

    up = np.stack([o["up"] for o in outs])                 # [C, 128, 16]
    usage = up[:, :, :E].sum(axis=(0, 1)) / T
    util = up[:, :, E:].sum(axis=(0, 1)) / T
    load_balance_loss = np.float32(np.mean((usage - 1.0 / E) ** 2))
    expert_utilization = util.astype(np.float32)

    dv = np.stack([o["dv"] for o in outs])                 # [C, 128, 28]
    d2 = dv.sum(axis=(0, 1))                               # [28]
    diversity_score = np.float32(np.mean(np.sqrt(d2)))

    return (final_output.astype(np.float32),
            routing_weights.astype(np.float32),
            expert_utilization,
            load_balance_loss,
            diversity_score)
